# revision 33
# baseline (speedup 1.0000x reference)
"""Causal self-attention (B=2, T=2048, C=1024, H=16) on 8 Trainium2 cores.

Sharding: tensor-parallel over heads (2 heads/core). Each core computes
q/k/v for its heads, causal attention, and its slice of the c_proj
contraction; the host sums the 8 partial projection outputs and adds
b_proj.

Device-side layout keeps activations transposed ([feat, tok]) so no
transposes of x/q/k are needed; v is transposed on-chip via a PE
transpose. Softmax runs over the partition axis of S^T: the denominator
comes for free from a ones-column appended to v in the P@V matmul.

v3: one global interleaved schedule. The attention i-loop is software-
pipelined (S(i+2) issues before PV(i) so the in-order PE never waits on
exp), and all non-attention matmuls (qkv groups, c_proj tiles, v
transposes) are woven between attention tiles as fillers. This keeps
the tensor engine gap-free, which both hides the exp/normalize chains
and keeps the PE DVFS ramp at full clock (idle gaps drop it to the mid
pstate where matmuls run ~2x slower). exp covers both heads in one
instruction per k-tile (shared 2-bank PSUM S tile); the softmax
reciprocal uses the fast approximate DVE op (the exact one is 3.3us);
the normalize partition-hop and the h1 row relocation run as sbuf-sbuf
DMAs on the otherwise-idle scalar ring instead of 1.9us GpSimd copies;
the normalize multiplies are deferred into the next block's slots so
the in-order DVE stream never blocks on the broadcast chain.
"""

import sys

try:
    import concourse  # noqa: F401
except ImportError:
    sys.path.insert(0, "/opt/trn_rl_repo")

import numpy as np
import ml_dtypes

import concourse.bacc as bacc
import concourse.mybir as mybir
import concourse.tile as tile
from concourse import bass_utils

B, T, C, H, NCORES = 2, 2048, 1024, 16, 8
BT = B * T                  # 4096 tokens total
HPC = H // NCORES           # 2 heads per core
D = C // H                  # 64 head dim
CS = HPC * D                # 128 per-core feature slice
QB = 512                    # q block (free dim per matmul)
KT = 128                    # k tile (partition dim of S^T)
NB = T // QB                # 4 q-blocks per batch
NKT = T // KT               # 16 k-tiles per batch
NCT = C // 128              # 8 contraction tiles over C
BF16 = mybir.dt.bfloat16
F32 = mybir.dt.float32
SCALE = 1.0 / np.sqrt(D)

_built = {}

# ---- tuning knobs ----
QKV_EVAC = "scalar"      # "scalar" (ACT) | "vector" (DVE)
FILL_COLS = 1280         # filler matmul columns injected per attention tile
FIRST_FILL = 3584        # bigger fill at slot 0 (covers prev block's tail)
PV_SPLIT = False          # split diag PV so unmasked cols skip the mask dep
PROJ_LAG = 2             # blocks between a tail and its proj units
MASK_ENG = "vector"      # "gpsimd" | "vector" engine for the causal mask
VN_EVAC = "vector"       # "gpsimd" | "vector" engine for trp->vn copies


def _build(repeat=1):
    key = ("nc", repeat, QKV_EVAC, FILL_COLS, FIRST_FILL, PV_SPLIT,
           PROJ_LAG, MASK_ENG, VN_EVAC)
    if key in _built:
        return _built[key]

    nc = bacc.Bacc("TRN2", target_bir_lowering=False, debug=False,
                   num_devices=NCORES)
    xT = nc.dram_tensor("xT", [C, BT], BF16, kind="ExternalInput")
    wqkv = nc.dram_tensor("wqkv", [C, 3 * CS], BF16, kind="ExternalInput")
    bqkv = nc.dram_tensor("bqkv", [3 * CS, 1], F32, kind="ExternalInput")
    wproj = nc.dram_tensor("wproj", [CS, C], BF16, kind="ExternalInput")
    outT = nc.dram_tensor("outT", [C, BT], BF16, kind="ExternalOutput")

    with tile.TileContext(nc) as tc:
        _emit(nc, tc, xT.ap(), wqkv.ap(), bqkv.ap(), wproj.ap(), outT.ap(),
              repeat=repeat)
    nc.compile()
    _built[key] = nc
    return nc


def _emit(nc, tc, xT, wqkv, bqkv, wproj, outT, repeat=1, dbg=None):
    from contextlib import ExitStack
    ctx = ExitStack()
    with ctx:
        constp = ctx.enter_context(tc.tile_pool(name="const", bufs=1))
        xp = ctx.enter_context(tc.tile_pool(name="x", bufs=1))
        wp = ctx.enter_context(tc.tile_pool(name="w", bufs=1))
        qkvp = ctx.enter_context(tc.tile_pool(name="qkv", bufs=1))
        vnp = ctx.enter_context(tc.tile_pool(name="vnat", bufs=1))
        ppool = ctx.enter_context(tc.tile_pool(name="pp", bufs=4))
        ypool = ctx.enter_context(tc.tile_pool(name="yt", bufs=1))
        osp = ctx.enter_context(tc.tile_pool(name="ostage", bufs=4))
        rpool = ctx.enter_context(tc.tile_pool(name="rec", bufs=2))
        # PSUM: psS 2x[128,1024] (4 banks) + psO 2x[65,512] (2) + psQ 2 = 8
        psS = ctx.enter_context(tc.tile_pool(name="psS", bufs=2,
                                             space="PSUM"))
        psO = ctx.enter_context(tc.tile_pool(name="psO", bufs=2,
                                             space="PSUM"))
        psQ = ctx.enter_context(tc.tile_pool(name="psQ", bufs=2,
                                             space="PSUM"))

        # ---- constants / weights / inputs ----
        w_sb = wp.tile([128, NCT, 3, CS], BF16)
        nc.sync.dma_start(
            w_sb[:],
            wqkv.rearrange("(a p) (m c) -> p a m c", p=128, m=3))
        wp_sb = wp.tile([128, C], BF16)             # W_proj slice [CS=128, C]
        nc.sync.dma_start(wp_sb[:], wproj[:, :])
        bias_sb = wp.tile([128, 3], F32)
        nc.sync.dma_start(bias_sb[:],
                          bqkv.rearrange("(m p) o -> p (m o)", p=128))

        zbias = constp.tile([128, 1], F32)         # explicit exp bias=0:
        nc.gpsimd.memset(zbias[:], 0.0)            # a float bias would pull
        # in a const-AP DMA that queues behind all input DMAs

        ident = constp.tile([128, 128], BF16)      # for PE transpose
        from concourse.masks import make_identity
        make_identity(nc, ident[:])

        # causal mask for the 128x128 diagonal blocks of S^T: keep k <= q
        mask = constp.tile([128, KT], BF16)
        nc.gpsimd.memset(mask[:], 1.0)
        nc.gpsimd.affine_select(
            out=mask[:], in_=mask[:],
            compare_op=mybir.AluOpType.is_ge,
            fill=0.0, base=0, pattern=[[1, KT]],
            channel_multiplier=-1)

        # xT c-tiles, loaded per (token-chunk, c-tile) for early start
        x_sb = xp.tile([128, NCT, BT], BF16)
        XC = 512
        for nn_ in range(BT // XC):
            for a in range(NCT):
                nc.sync.dma_start(
                    x_sb[:, a, nn_ * XC:(nn_ + 1) * XC],
                    xT[a * 128:(a + 1) * 128, nn_ * XC:(nn_ + 1) * XC])

        # qkvT activations, [feat 128, tok] each; v produced transposed too
        q_sb = qkvp.tile([128, BT], BF16, tag="q")
        k_sb = qkvp.tile([128, BT], BF16, tag="k")
        vT_sb = qkvp.tile([128, BT], BF16, tag="vT")
        qkv_dst = [q_sb, k_sb, vT_sb]

        # v natural layout per (b, h, ktile): [tok 128, slot 128] with
        # cols [v(64) | ones | pad]: the ones column makes the P@V matmul
        # also emit the softmax denominator (O' at psum partitions 0:64,
        # denom at 64).
        vn_sb = vnp.tile([128, B, HPC, NKT, 128], BF16)
        nc.gpsimd.memset(vn_sb[:, :, :, :, 64:65], 1.0)

        yT_sb = ypool.tile([128, BT], BF16)         # per-core y^T slice

        # ---------- filler units: generators yielding (cols, emit_fn) ----
        def unit_qkv_group(b, n, m):
            tb = b * T
            ps = psQ.tile([128, QB], F32, tag="psQ", name="qkvps")
            for a in range(NCT):
                def mm(a=a, ps=ps):
                    nc.tensor.matmul(
                        ps[:], w_sb[:, a, m, :],
                        x_sb[:, a, tb + n * QB: tb + (n + 1) * QB],
                        start=(a == 0), stop=(a == NCT - 1))
                yield QB, mm
            def evac(ps=ps):
                dst = qkv_dst[m][:, tb + n * QB: tb + (n + 1) * QB]
                if QKV_EVAC == "scalar":
                    nc.scalar.add(dst, ps[:], bias_sb[:, m:m + 1])
                else:
                    nc.vector.tensor_scalar_add(dst, ps[:],
                                                bias_sb[:, m:m + 1])
            yield 0, evac

        def unit_trp(b, i):
            tb = b * T
            trp = psQ.tile([128, KT], BF16, tag="psQ", name="trp")
            def mm(trp=trp):
                nc.tensor.transpose(
                    trp[:], vT_sb[:, tb + i * KT: tb + (i + 1) * KT],
                    ident[:])
            yield KT, mm
            def evac(trp=trp):
                eng = nc.gpsimd if VN_EVAC == "gpsimd" else nc.vector
                for h in range(HPC):
                    eng.tensor_copy(vn_sb[:, b, h, i, 0:64],
                                    trp[:, h * 64:(h + 1) * 64])
            yield 0, evac

        def unit_proj_oc2(b, j, oc2):
            # two output tiles per unit: the second matmul chains on the
            # PE right behind the first (hides the per-matmul latency)
            # and the pair shares one wide store DMA.
            tb = b * T
            ost = osp.tile([128, 2 * QB], BF16, tag="ostage", name="ost")
            pos = []
            for k in range(2):
                po = psQ.tile([128, QB], F32, tag="psQ", name="po")
                pos.append(po)
                def mm(po=po, oc=oc2 * 2 + k):
                    nc.tensor.matmul(
                        po[:], wp_sb[:, oc * 128:(oc + 1) * 128],
                        yT_sb[:, tb + j * QB: tb + (j + 1) * QB],
                        start=True, stop=True)
                yield QB, mm
            def evac():
                for k in range(2):
                    nc.vector.tensor_copy(ost[:, k * QB:(k + 1) * QB],
                                          pos[k][:])
                nc.sync.dma_start(
                    outT[oc2 * 256:(oc2 + 1) * 256,
                         tb + j * QB: tb + (j + 1) * QB]
                    .rearrange("(t p) q -> p t q", t=2),
                    ost[:].rearrange("p (t q) -> p t q", t=2))
            yield 0, evac



        # ---------- the filler scheduler ----------
        queue = []          # list of (key, generator)
        active = [None]     # currently-draining (key, generator)

        def push(key, gen):
            queue.append((key, gen))

        def drain_unit(gen):
            for _cols, fn in gen:
                fn()

        def force(pred):
            if active[0] is not None and pred(active[0][0]):
                drain_unit(active[0][1])
                active[0] = None
            keep = []
            for key, gen in queue:
                if pred(key):
                    drain_unit(gen)
                else:
                    keep.append((key, gen))
            queue[:] = keep

        def fill(budget):
            """Emit ~budget streamed columns of filler; at most one unit
            COMPLETES per call (keeps psQ evac latency off the PE)."""
            while budget > 0:
                if active[0] is None:
                    if not queue:
                        return
                    active[0] = queue.pop(0)
                gen = active[0][1]
                for cols, fn in gen:
                    fn()
                    budget -= cols
                    if budget <= 0:
                        return
                # unit exhausted: stop this fill slot
                active[0] = None
                return

        # ---------- attention block (software-pipelined i-loop) ----------
        def attn_block(b, j, tail_steps):
            tb = b * T
            nkt_j = 4 * (j + 1)
            ops = [psO.tile([65, QB], F32, tag="psO", name=f"op{h}")
                   for h in range(HPC)]
            sts = {}
            pps = {}

            def c0_of(i):
                return 0 if i < 4 * j else KT * (i - 4 * j)

            def emit_S(i):
                c0 = c0_of(i)
                st = psS.tile([128, 2 * QB], F32, tag="psS", name="s")
                sts[i] = st
                for h in range(HPC):
                    hs = h * 64
                    nc.tensor.matmul(
                        st[:, h * QB + c0: h * QB + QB],
                        k_sb[hs:hs + 64, tb + i * KT: tb + (i + 1) * KT],
                        q_sb[hs:hs + 64,
                             tb + j * QB + c0: tb + (j + 1) * QB],
                        start=True, stop=True)

            def emit_exp(i):
                c0 = c0_of(i)
                st = sts[i]
                p = ppool.tile([128, 2 * QB], BF16, tag="pp", name="pp")
                pps[i] = p
                if c0 == 0:
                    nc.scalar.activation(
                        p[:, 0:2 * QB], st[:, 0:2 * QB],
                        mybir.ActivationFunctionType.Exp,
                        bias=zbias[:, 0:1], scale=SCALE)
                else:
                    for h in range(HPC):
                        nc.scalar.activation(
                            p[:, h * QB + c0: (h + 1) * QB],
                            st[:, h * QB + c0: h * QB + QB],
                            mybir.ActivationFunctionType.Exp,
                            bias=zbias[:, 0:1], scale=SCALE)

            def emit_mask(i):
                c0 = c0_of(i)
                p = pps[i]
                eng = nc.gpsimd if MASK_ENG == "gpsimd" else nc.vector
                for h in range(HPC):
                    po = h * QB + c0
                    eng.tensor_mul(
                        p[:, po:po + KT], p[:, po:po + KT], mask[:])

            def emit_PV(i):
                c0 = c0_of(i)
                w = QB - c0
                p = pps[i]
                diag = i >= 4 * j
                for h in range(HPC):
                    po = h * QB
                    if diag and PV_SPLIT and w > KT:
                        nc.tensor.matmul(
                            ops[h][0:65, c0 + KT:QB],
                            vn_sb[:, b, h, i, 0:65],
                            p[:, po + c0 + KT: po + QB],
                            start=(i == 0), stop=False)
                        nc.tensor.matmul(
                            ops[h][0:65, c0:c0 + KT],
                            vn_sb[:, b, h, i, 0:65],
                            p[:, po + c0: po + c0 + KT],
                            start=False, stop=(i == nkt_j - 1))
                    else:
                        nc.tensor.matmul(
                            ops[h][0:65, c0:QB],
                            vn_sb[:, b, h, i, 0:65],
                            p[:, po + c0: po + QB],
                            start=(i == 0), stop=(i == nkt_j - 1))

            emit_S(0); emit_exp(0)
            if nkt_j > 1:
                emit_S(1); emit_exp(1)
            for i in range(nkt_j):
                fill(FIRST_FILL if i == 0 else FILL_COLS)
                if 1 <= i <= len(tail_steps):
                    tail_steps[i - 1]()          # prev block's tail piece
                if i >= 4 * j:
                    emit_mask(i)
                emit_PV(i)
                if i + 2 < nkt_j:
                    emit_S(i + 2)
                    emit_exp(i + 2)
                sts.pop(i, None)
                pps.pop(i, None)
            for s in tail_steps[nkt_j - 1:]:     # leftovers (short blocks)
                s()

            # Pipelined tail. psO has only 2 buffers, so everything that
            # reads the O' accumulators is emitted inline: both heads'
            # denominator rows and O' bodies staged to SBUF (DVE, ~2.6us).
            # The rest of the chain defers into the next block's slots.
            # The 1024-wide reciprocal is reshaped to [64,16] via DMA so
            # the exact DVE reciprocal (cost ~ free size) drops from
            # 6.6us to ~0.2us. Tail DMAs ride the sync ring: a dma_start
            # doorbell costs ~600ns ON its issuing engine, which made the
            # scalar (ACT) queue a bottleneck when they lived there.
            blk = slice(tb + j * QB, tb + (j + 1) * QB)
            den = rpool.tile([65, 2 * QB], F32, tag="den", name="den")
            oc0 = rpool.tile([64, QB], F32, tag="oc0", name="oc0")
            oc1 = rpool.tile([64, QB], F32, tag="oc1", name="oc1")
            nc.vector.tensor_copy(den[64:65, 0:QB], ops[0][64:65, :])
            nc.vector.tensor_copy(den[64:65, QB:2 * QB], ops[1][64:65, :])
            nc.vector.tensor_copy(oc0[0:64, :], ops[0][0:64, :])
            nc.vector.tensor_copy(oc1[0:64, :], ops[1][0:64, :])

            r8a = rpool.tile([64, 16], F32, tag="r8a", name="r8a")
            r8b = rpool.tile([64, 16], F32, tag="r8b", name="r8b")
            rc = rpool.tile([1, 2 * QB], F32, tag="rc", name="rc")
            rb = rpool.tile([64, 2 * QB], F32, tag="rb", name="rb")
            ytmp = rpool.tile([64, QB], BF16, tag="ytmp", name="ytmp")

            def s0():
                nc.sync.dma_start(r8a[0:64, 0:16], den[64:65, :])

            def s1():
                nc.vector.reciprocal(r8b[0:64, 0:16], r8a[0:64, 0:16])

            def s2():
                nc.sync.dma_start(rc[0:1, :], r8b[0:64, 0:16])

            def s3():
                nc.gpsimd.partition_broadcast(rb[0:64, :], rc[0:1, :])

            def s4():
                nc.vector.tensor_mul(yT_sb[0:64, blk], oc0[0:64, :],
                                     rb[0:64, 0:QB])
                nc.vector.tensor_mul(ytmp[0:64, :], oc1[0:64, :],
                                     rb[0:64, QB:2 * QB])
                nc.sync.dma_start(yT_sb[64:128, blk], ytmp[0:64, :])

            return [s0, s1, s2, s3, s4]

        # ---------- global schedule ----------
        def seed_units(b):
            for n in range(NB):
                for m in range(3):
                    push(("qkv", b, n, m), unit_qkv_group(b, n, m))
                for i in range(4 * n, 4 * n + 4):
                    push(("trp", b, i), unit_trp(b, i))

        def force_prereqs(b, j):
            def pred(key):
                if key[0] == "qkv" and key[1] == b and key[2] <= j:
                    return True
                if key[0] == "trp" and key[1] == b and key[2] < 4 * (j + 1):
                    return True
                return False
            force(pred)

        seed_units(0)
        seed_units(1)
        pending_proj = []            # [(due_step, key, gen), ...]
        tail_steps = []
        step = 0
        for rep in range(repeat):
            for b in range(B):
                for j in range(NB):
                    # release proj units that are due
                    due = [p for p in pending_proj if p[0] <= step]
                    pending_proj[:] = [p for p in pending_proj
                                       if p[0] > step]
                    for _d, key, gen in due:
                        push(key, gen)
                    force_prereqs(b, j)
                    tail_steps = attn_block(b, j, tail_steps)
                    for oc2 in range(NCT // 2):
                        pending_proj.append(
                            (step + PROJ_LAG,
                             ("proj", b, j, oc2),
                             unit_proj_oc2(b, j, oc2)))
                    step += 1
                if rep + 1 < repeat:
                    seed_units(b)
        for s in tail_steps:
            s()
        for _d, key, gen in pending_proj:
            push(key, gen)
        force(lambda key: True)

        if dbg is not None:
            nc.sync.dma_start(dbg["q"].ap(), q_sb[:])
            nc.sync.dma_start(dbg["k"].ap(), k_sb[:])
            nc.sync.dma_start(dbg["vT"].ap(), vT_sb[:])
            nc.sync.dma_start(
                dbg["vn"].ap(),
                vn_sb[:].rearrange("p a b c d -> p (a b c d)"))
            nc.sync.dma_start(dbg["y"].ap(), yT_sb[:])
            nc.sync.dma_start(dbg["mask"].ap(), mask[:])


def _host_inputs(x, W_attn, b_attn):
    bf = ml_dtypes.bfloat16
    xTh = np.ascontiguousarray(
        x.reshape(BT, C).T.astype(bf))
    in_maps = []
    for c in range(NCORES):
        lo = c * CS
        wq = W_attn[:, lo:lo + CS]
        wk = W_attn[:, C + lo: C + lo + CS]
        wv = W_attn[:, 2 * C + lo: 2 * C + lo + CS]
        wqkv = np.ascontiguousarray(
            np.concatenate([wq, wk, wv], axis=1).astype(bf))
        bq = np.concatenate([b_attn[lo:lo + CS],
                             b_attn[C + lo: C + lo + CS],
                             b_attn[2 * C + lo: 2 * C + lo + CS]])
        bqkvh = np.ascontiguousarray(
            bq.reshape(3 * CS, 1).astype(np.float32))
        in_maps.append({"xT": xTh, "wqkv": wqkv, "bqkv": bqkvh})
    return in_maps


def kernel(x, W_attn, b_attn, W_proj, b_proj):
    x = np.asarray(x, np.float32)
    W_attn = np.asarray(W_attn, np.float32)
    b_attn = np.asarray(b_attn, np.float32)
    W_proj = np.asarray(W_proj, np.float32)
    b_proj = np.asarray(b_proj, np.float32)

    nc = _build()
    in_maps = _host_inputs(x, W_attn, b_attn)
    bf = ml_dtypes.bfloat16
    for c in range(NCORES):
        in_maps[c]["wproj"] = np.ascontiguousarray(
            W_proj[c * CS:(c + 1) * CS, :].astype(bf))

    res = bass_utils.run_bass_kernel_spmd(
        nc, in_maps, core_ids=list(range(NCORES)))
    acc = np.zeros((C, BT), np.float64)
    for c in range(NCORES):
        acc += res.results[c]["outT"].astype(np.float64)
    out = acc.T.astype(np.float32) + b_proj[None, :]
    return out.reshape(B, T, C)


# revision 34
# speedup vs baseline: 1.0571x; 1.0571x over previous
"""Causal self-attention (B=2, T=2048, C=1024, H=16) on 8 Trainium2 cores.

Sharding: tensor-parallel over heads (2 heads/core). Each core computes
q/k/v for its heads, causal attention, and its slice of the c_proj
contraction; the host sums the 8 partial projection outputs and adds
b_proj.

Device-side layout keeps activations transposed ([feat, tok]) so no
transposes of x/q/k are needed; v is transposed on-chip via a PE
transpose. Softmax runs over the partition axis of S^T: the denominator
comes for free from a ones-column appended to v in the P@V matmul.

v3: one global interleaved schedule. The attention i-loop is software-
pipelined (S(i+2) issues before PV(i) so the in-order PE never waits on
exp), and all non-attention matmuls (qkv groups, c_proj tiles, v
transposes) are woven between attention tiles as fillers. This keeps
the tensor engine gap-free, which both hides the exp/normalize chains
and keeps the PE DVFS ramp at full clock (idle gaps drop it to the mid
pstate where matmuls run ~2x slower). exp covers both heads in one
instruction per k-tile (shared 2-bank PSUM S tile); the softmax
reciprocal uses the fast approximate DVE op (the exact one is 3.3us);
the normalize partition-hop and the h1 row relocation run as sbuf-sbuf
DMAs on the otherwise-idle scalar ring instead of 1.9us GpSimd copies;
the normalize multiplies are deferred into the next block's slots so
the in-order DVE stream never blocks on the broadcast chain.
"""

import sys

try:
    import concourse  # noqa: F401
except ImportError:
    sys.path.insert(0, "/opt/trn_rl_repo")

import numpy as np
import ml_dtypes

import concourse.bacc as bacc
import concourse.mybir as mybir
import concourse.tile as tile
from concourse import bass_utils

B, T, C, H, NCORES = 2, 2048, 1024, 16, 8
BT = B * T                  # 4096 tokens total
HPC = H // NCORES           # 2 heads per core
D = C // H                  # 64 head dim
CS = HPC * D                # 128 per-core feature slice
QB = 512                    # q block (free dim per matmul)
KT = 128                    # k tile (partition dim of S^T)
NB = T // QB                # 4 q-blocks per batch
NKT = T // KT               # 16 k-tiles per batch
NCT = C // 128              # 8 contraction tiles over C
BF16 = mybir.dt.bfloat16
F32 = mybir.dt.float32
SCALE = 1.0 / np.sqrt(D)

_built = {}

# ---- tuning knobs ----
QKV_EVAC = "scalar"      # "scalar" (ACT) | "vector" (DVE)
FILL_COLS = 1280         # filler matmul columns injected per attention tile
FIRST_FILL = 3584        # bigger fill at slot 0 (covers prev block's tail)
PV_SPLIT = False          # split diag PV so unmasked cols skip the mask dep
PROJ_LAG = 2             # blocks between a tail and its proj units
MASK_ENG = "vector"      # "gpsimd" | "vector" engine for the causal mask
VN_EVAC = "vector"       # "gpsimd" | "vector" engine for trp->vn copies


def _build(repeat=1):
    key = ("nc", repeat, QKV_EVAC, FILL_COLS, FIRST_FILL, PV_SPLIT,
           PROJ_LAG, MASK_ENG, VN_EVAC)
    if key in _built:
        return _built[key]

    nc = bacc.Bacc("TRN2", target_bir_lowering=False, debug=False,
                   num_devices=NCORES)
    xT = nc.dram_tensor("xT", [C, BT], BF16, kind="ExternalInput")
    wqkv = nc.dram_tensor("wqkv", [C, 3 * CS], BF16, kind="ExternalInput")
    bqkv = nc.dram_tensor("bqkv", [3 * CS, 1], F32, kind="ExternalInput")
    wproj = nc.dram_tensor("wproj", [CS, C], BF16, kind="ExternalInput")
    outT = nc.dram_tensor("outT", [C, BT], BF16, kind="ExternalOutput")

    with tile.TileContext(nc) as tc:
        _emit(nc, tc, xT.ap(), wqkv.ap(), bqkv.ap(), wproj.ap(), outT.ap(),
              repeat=repeat)
    nc.compile()
    _built[key] = nc
    return nc


def _emit(nc, tc, xT, wqkv, bqkv, wproj, outT, repeat=1, dbg=None):
    from contextlib import ExitStack
    ctx = ExitStack()
    with ctx:
        constp = ctx.enter_context(tc.tile_pool(name="const", bufs=1))
        xp = ctx.enter_context(tc.tile_pool(name="x", bufs=1))
        wp = ctx.enter_context(tc.tile_pool(name="w", bufs=1))
        qkvp = ctx.enter_context(tc.tile_pool(name="qkv", bufs=1))
        vnp = ctx.enter_context(tc.tile_pool(name="vnat", bufs=1))
        ppool = ctx.enter_context(tc.tile_pool(name="pp", bufs=4))
        ypool = ctx.enter_context(tc.tile_pool(name="yt", bufs=1))
        osp = ctx.enter_context(tc.tile_pool(name="ostage", bufs=4))
        rpool = ctx.enter_context(tc.tile_pool(name="rec", bufs=2))
        # PSUM: psS 2x[128,1024] (4 banks) + psO 2x[65,512] (2) + psQ 2 = 8
        psS = ctx.enter_context(tc.tile_pool(name="psS", bufs=2,
                                             space="PSUM"))
        psO = ctx.enter_context(tc.tile_pool(name="psO", bufs=2,
                                             space="PSUM"))
        psQ = ctx.enter_context(tc.tile_pool(name="psQ", bufs=2,
                                             space="PSUM"))

        # ---- constants / weights / inputs ----
        w_sb = wp.tile([128, NCT, 3, CS], BF16)
        nc.sync.dma_start(
            w_sb[:],
            wqkv.rearrange("(a p) (m c) -> p a m c", p=128, m=3))
        wp_sb = wp.tile([128, C], BF16)             # W_proj slice [CS=128, C]
        nc.sync.dma_start(wp_sb[:], wproj[:, :])
        bias_sb = wp.tile([128, 3], F32)
        nc.sync.dma_start(bias_sb[:],
                          bqkv.rearrange("(m p) o -> p (m o)", p=128))

        zbias = constp.tile([128, 1], F32)         # explicit exp bias=0:
        nc.gpsimd.memset(zbias[:], 0.0)            # a float bias would pull
        # in a const-AP DMA that queues behind all input DMAs

        ident = constp.tile([128, 128], BF16)      # for PE transpose
        from concourse.masks import make_identity
        make_identity(nc, ident[:])

        # causal mask for the 128x128 diagonal blocks of S^T: keep k <= q
        mask = constp.tile([128, KT], BF16)
        nc.gpsimd.memset(mask[:], 1.0)
        nc.gpsimd.affine_select(
            out=mask[:], in_=mask[:],
            compare_op=mybir.AluOpType.is_ge,
            fill=0.0, base=0, pattern=[[1, KT]],
            channel_multiplier=-1)

        # xT c-tiles, loaded per (token-chunk, c-tile) for early start
        x_sb = xp.tile([128, NCT, BT], BF16)
        XC = 512
        for nn_ in range(BT // XC):
            for a in range(NCT):
                nc.sync.dma_start(
                    x_sb[:, a, nn_ * XC:(nn_ + 1) * XC],
                    xT[a * 128:(a + 1) * 128, nn_ * XC:(nn_ + 1) * XC])

        # qkvT activations, [feat 128, tok] each; v produced transposed too
        q_sb = qkvp.tile([128, BT], BF16, tag="q")
        k_sb = qkvp.tile([128, BT], BF16, tag="k")
        vT_sb = qkvp.tile([128, BT], BF16, tag="vT")
        qkv_dst = [q_sb, k_sb, vT_sb]

        # v natural layout per (b, h, ktile): [tok 128, slot 128] with
        # cols [v(64) | ones | pad]: the ones column makes the P@V matmul
        # also emit the softmax denominator (O' at psum partitions 0:64,
        # denom at 64).
        vn_sb = vnp.tile([128, B, HPC, NKT, 128], BF16)
        nc.gpsimd.memset(vn_sb[:, :, :, :, 64:65], 1.0)

        yT_sb = ypool.tile([128, BT], BF16)         # per-core y^T slice

        # ---------- filler units: generators yielding (cols, emit_fn) ----
        def unit_qkv_group(b, n, m):
            tb = b * T
            ps = psQ.tile([128, QB], F32, tag="psQ", name="qkvps")
            for a in range(NCT):
                def mm(a=a, ps=ps):
                    nc.tensor.matmul(
                        ps[:], w_sb[:, a, m, :],
                        x_sb[:, a, tb + n * QB: tb + (n + 1) * QB],
                        start=(a == 0), stop=(a == NCT - 1))
                yield QB, mm
            def evac(ps=ps):
                dst = qkv_dst[m][:, tb + n * QB: tb + (n + 1) * QB]
                if QKV_EVAC == "scalar":
                    nc.scalar.add(dst, ps[:], bias_sb[:, m:m + 1])
                else:
                    nc.vector.tensor_scalar_add(dst, ps[:],
                                                bias_sb[:, m:m + 1])
            yield 0, evac

        def unit_trp(b, i):
            tb = b * T
            trp = psQ.tile([128, KT], BF16, tag="psQ", name="trp")
            def mm(trp=trp):
                nc.tensor.transpose(
                    trp[:], vT_sb[:, tb + i * KT: tb + (i + 1) * KT],
                    ident[:])
            yield KT, mm
            def evac(trp=trp):
                eng = nc.gpsimd if VN_EVAC == "gpsimd" else nc.vector
                for h in range(HPC):
                    eng.tensor_copy(vn_sb[:, b, h, i, 0:64],
                                    trp[:, h * 64:(h + 1) * 64])
            yield 0, evac

        def unit_proj_oc(b, j, oc):
            tb = b * T
            po = psQ.tile([128, QB], F32, tag="psQ", name="po")
            ost = osp.tile([128, QB], BF16, tag="ostage", name="ost")
            def mm(po=po):
                nc.tensor.matmul(
                    po[:], wp_sb[:, oc * 128:(oc + 1) * 128],
                    yT_sb[:, tb + j * QB: tb + (j + 1) * QB],
                    start=True, stop=True)
            yield QB, mm
            def evac(po=po, ost=ost):
                nc.vector.tensor_copy(ost[:], po[:])
                nc.sync.dma_start(
                    outT[oc * 128:(oc + 1) * 128,
                         tb + j * QB: tb + (j + 1) * QB],
                    ost[:])
            yield 0, evac



        # ---------- the filler scheduler ----------
        queue = []          # list of (key, generator)
        active = [None]     # currently-draining (key, generator)

        def push(key, gen):
            queue.append((key, gen))

        def drain_unit(gen):
            for _cols, fn in gen:
                fn()

        def force(pred):
            if active[0] is not None and pred(active[0][0]):
                drain_unit(active[0][1])
                active[0] = None
            keep = []
            for key, gen in queue:
                if pred(key):
                    drain_unit(gen)
                else:
                    keep.append((key, gen))
            queue[:] = keep

        def fill(budget):
            """Emit ~budget streamed columns of filler; at most one unit
            COMPLETES per call (keeps psQ evac latency off the PE)."""
            while budget > 0:
                if active[0] is None:
                    if not queue:
                        return
                    active[0] = queue.pop(0)
                gen = active[0][1]
                for cols, fn in gen:
                    fn()
                    budget -= cols
                    if budget <= 0:
                        return
                # unit exhausted: stop this fill slot
                active[0] = None
                return

        # ---------- attention block (software-pipelined i-loop) ----------
        def attn_block(b, j, tail_steps):
            tb = b * T
            nkt_j = 4 * (j + 1)
            ops = [psO.tile([65, QB], F32, tag="psO", name=f"op{h}")
                   for h in range(HPC)]
            sts = {}
            pps = {}

            def c0_of(i):
                return 0 if i < 4 * j else KT * (i - 4 * j)

            def emit_S(i):
                c0 = c0_of(i)
                st = psS.tile([128, 2 * QB], F32, tag="psS", name="s")
                sts[i] = st
                for h in range(HPC):
                    hs = h * 64
                    nc.tensor.matmul(
                        st[:, h * QB + c0: h * QB + QB],
                        k_sb[hs:hs + 64, tb + i * KT: tb + (i + 1) * KT],
                        q_sb[hs:hs + 64,
                             tb + j * QB + c0: tb + (j + 1) * QB],
                        start=True, stop=True)

            def emit_exp(i):
                c0 = c0_of(i)
                st = sts[i]
                p = ppool.tile([128, 2 * QB], BF16, tag="pp", name="pp")
                pps[i] = p
                if c0 == 0:
                    nc.scalar.activation(
                        p[:, 0:2 * QB], st[:, 0:2 * QB],
                        mybir.ActivationFunctionType.Exp,
                        bias=zbias[:, 0:1], scale=SCALE)
                else:
                    for h in range(HPC):
                        nc.scalar.activation(
                            p[:, h * QB + c0: (h + 1) * QB],
                            st[:, h * QB + c0: h * QB + QB],
                            mybir.ActivationFunctionType.Exp,
                            bias=zbias[:, 0:1], scale=SCALE)

            def emit_mask(i):
                c0 = c0_of(i)
                p = pps[i]
                eng = nc.gpsimd if MASK_ENG == "gpsimd" else nc.vector
                for h in range(HPC):
                    po = h * QB + c0
                    eng.tensor_mul(
                        p[:, po:po + KT], p[:, po:po + KT], mask[:])

            def emit_PV(i):
                c0 = c0_of(i)
                w = QB - c0
                p = pps[i]
                diag = i >= 4 * j
                for h in range(HPC):
                    po = h * QB
                    if diag and PV_SPLIT and w > KT:
                        nc.tensor.matmul(
                            ops[h][0:65, c0 + KT:QB],
                            vn_sb[:, b, h, i, 0:65],
                            p[:, po + c0 + KT: po + QB],
                            start=(i == 0), stop=False)
                        nc.tensor.matmul(
                            ops[h][0:65, c0:c0 + KT],
                            vn_sb[:, b, h, i, 0:65],
                            p[:, po + c0: po + c0 + KT],
                            start=False, stop=(i == nkt_j - 1))
                    else:
                        nc.tensor.matmul(
                            ops[h][0:65, c0:QB],
                            vn_sb[:, b, h, i, 0:65],
                            p[:, po + c0: po + QB],
                            start=(i == 0), stop=(i == nkt_j - 1))

            emit_S(0); emit_exp(0)
            if nkt_j > 1:
                emit_S(1); emit_exp(1)
            for i in range(nkt_j):
                fill(FIRST_FILL if i == 0 else FILL_COLS)
                if 1 <= i <= len(tail_steps):
                    tail_steps[i - 1]()          # prev block's tail piece
                if i >= 4 * j:
                    emit_mask(i)
                emit_PV(i)
                if i + 2 < nkt_j:
                    emit_S(i + 2)
                    emit_exp(i + 2)
                sts.pop(i, None)
                pps.pop(i, None)
            for s in tail_steps[nkt_j - 1:]:     # leftovers (short blocks)
                s()

            # Pipelined tail. psO has only 2 buffers, so everything that
            # reads the O' accumulators is emitted inline: both heads'
            # denominator rows and O' bodies staged to SBUF (DVE, ~2.6us).
            # The rest of the chain defers into the next block's slots.
            # The 1024-wide reciprocal is reshaped to [64,16] via DMA so
            # the exact DVE reciprocal (cost ~ free size) drops from
            # 6.6us to ~0.2us. Tail DMAs ride the sync ring: a dma_start
            # doorbell costs ~600ns ON its issuing engine, which made the
            # scalar (ACT) queue a bottleneck when they lived there.
            blk = slice(tb + j * QB, tb + (j + 1) * QB)
            den = rpool.tile([65, 2 * QB], F32, tag="den", name="den")
            oc0 = rpool.tile([64, QB], F32, tag="oc0", name="oc0")
            oc1 = rpool.tile([64, QB], F32, tag="oc1", name="oc1")
            nc.vector.tensor_copy(den[64:65, 0:QB], ops[0][64:65, :])
            nc.vector.tensor_copy(den[64:65, QB:2 * QB], ops[1][64:65, :])
            nc.vector.tensor_copy(oc0[0:64, :], ops[0][0:64, :])
            nc.vector.tensor_copy(oc1[0:64, :], ops[1][0:64, :])

            r8a = rpool.tile([64, 16], F32, tag="r8a", name="r8a")
            r8b = rpool.tile([64, 16], F32, tag="r8b", name="r8b")
            rc = rpool.tile([1, 2 * QB], F32, tag="rc", name="rc")
            rb = rpool.tile([64, 2 * QB], F32, tag="rb", name="rb")
            ytmp = rpool.tile([64, QB], BF16, tag="ytmp", name="ytmp")

            def s0():
                nc.sync.dma_start(r8a[0:64, 0:16], den[64:65, :])

            def s1():
                nc.vector.reciprocal(r8b[0:64, 0:16], r8a[0:64, 0:16])

            def s2():
                nc.sync.dma_start(rc[0:1, :], r8b[0:64, 0:16])

            def s3():
                nc.gpsimd.partition_broadcast(rb[0:64, :], rc[0:1, :])

            def s4():
                nc.vector.tensor_mul(yT_sb[0:64, blk], oc0[0:64, :],
                                     rb[0:64, 0:QB])
                nc.vector.tensor_mul(ytmp[0:64, :], oc1[0:64, :],
                                     rb[0:64, QB:2 * QB])
                nc.sync.dma_start(yT_sb[64:128, blk], ytmp[0:64, :])

            return [s0, s1, s2, s3, s4]

        # ---------- global schedule ----------
        def seed_units(b):
            for n in range(NB):
                for m in range(3):
                    push(("qkv", b, n, m), unit_qkv_group(b, n, m))
                for i in range(4 * n, 4 * n + 4):
                    push(("trp", b, i), unit_trp(b, i))

        def force_prereqs(b, j):
            def pred(key):
                if key[0] == "qkv" and key[1] == b and key[2] <= j:
                    return True
                if key[0] == "trp" and key[1] == b and key[2] < 4 * (j + 1):
                    return True
                return False
            force(pred)

        seed_units(0)
        seed_units(1)
        pending_proj = []            # [(due_step, key, gen), ...]
        tail_steps = []
        step = 0
        for rep in range(repeat):
            for b in range(B):
                for j in range(NB):
                    # release proj units that are due
                    due = [p for p in pending_proj if p[0] <= step]
                    pending_proj[:] = [p for p in pending_proj
                                       if p[0] > step]
                    for _d, key, gen in due:
                        push(key, gen)
                    force_prereqs(b, j)
                    tail_steps = attn_block(b, j, tail_steps)
                    for oc in range(NCT):
                        pending_proj.append(
                            (step + PROJ_LAG,
                             ("proj", b, j, oc), unit_proj_oc(b, j, oc)))
                    step += 1
                if rep + 1 < repeat:
                    seed_units(b)
        for s in tail_steps:
            s()
        for _d, key, gen in pending_proj:
            push(key, gen)
        force(lambda key: True)

        if dbg is not None:
            nc.sync.dma_start(dbg["q"].ap(), q_sb[:])
            nc.sync.dma_start(dbg["k"].ap(), k_sb[:])
            nc.sync.dma_start(dbg["vT"].ap(), vT_sb[:])
            nc.sync.dma_start(
                dbg["vn"].ap(),
                vn_sb[:].rearrange("p a b c d -> p (a b c d)"))
            nc.sync.dma_start(dbg["y"].ap(), yT_sb[:])
            nc.sync.dma_start(dbg["mask"].ap(), mask[:])


def _host_inputs(x, W_attn, b_attn):
    bf = ml_dtypes.bfloat16
    xTh = np.ascontiguousarray(
        x.reshape(BT, C).T.astype(bf))
    in_maps = []
    for c in range(NCORES):
        lo = c * CS
        wq = W_attn[:, lo:lo + CS]
        wk = W_attn[:, C + lo: C + lo + CS]
        wv = W_attn[:, 2 * C + lo: 2 * C + lo + CS]
        wqkv = np.ascontiguousarray(
            np.concatenate([wq, wk, wv], axis=1).astype(bf))
        bq = np.concatenate([b_attn[lo:lo + CS],
                             b_attn[C + lo: C + lo + CS],
                             b_attn[2 * C + lo: 2 * C + lo + CS]])
        bqkvh = np.ascontiguousarray(
            bq.reshape(3 * CS, 1).astype(np.float32))
        in_maps.append({"xT": xTh, "wqkv": wqkv, "bqkv": bqkvh})
    return in_maps


def kernel(x, W_attn, b_attn, W_proj, b_proj):
    x = np.asarray(x, np.float32)
    W_attn = np.asarray(W_attn, np.float32)
    b_attn = np.asarray(b_attn, np.float32)
    W_proj = np.asarray(W_proj, np.float32)
    b_proj = np.asarray(b_proj, np.float32)

    nc = _build()
    in_maps = _host_inputs(x, W_attn, b_attn)
    bf = ml_dtypes.bfloat16
    for c in range(NCORES):
        in_maps[c]["wproj"] = np.ascontiguousarray(
            W_proj[c * CS:(c + 1) * CS, :].astype(bf))

    res = bass_utils.run_bass_kernel_spmd(
        nc, in_maps, core_ids=list(range(NCORES)))
    acc = np.zeros((C, BT), np.float64)
    for c in range(NCORES):
        acc += res.results[c]["outT"].astype(np.float64)
    out = acc.T.astype(np.float32) + b_proj[None, :]
    return out.reshape(B, T, C)
